# revision 12
# baseline (speedup 1.0000x reference)
"""Trainium2 Bass kernel for nn_DigitConvolutionalModel (dense CNN -> MLP).

Pure data parallel over 8 NeuronCores (2048 samples each). The 3x3 conv is
linear, so the host folds it into the first FC layer (W1e = C @ w1.T), making
the whole network a 4-layer MLP computed in transposed orientation (features
on partitions, batch on the free dim) in fp16 (psum fp32, ~5e-4 rel err):

    outT = w4t.T @ relu(w3t.T @ relu(w2t.T @ relu(W1e.T @ xT + b1) + b2) + b3) + b4

Raw bass with manual semaphores. W1 and x live in ONE fused SBUF region so
each DMA piece is a single contiguous transfer carrying both weights and
data - 5 pieces total ([w1a|t0a], [w1b|t0b], t1, t2, t3) which minimizes the
~1us per-piece completion-receipt stall on the SDMA engines. Pieces are
spread across both HWDGE rings in need order. The tensor engine opens with a
full-array warmup burst sized to the DMA-bound L1 start so the HAM
clock-gate reaches 8/8 before real work, and dummy matmuls bridge the
x-tile waits so the PE never re-throttles. The two h1-relu halves run in
parallel on ACT (m0) and DVE (m1).

PE op order (A=L1, B=L2, C=L3, D=L4):
  A0 A1 B0 A2 C0 B1 A3 D0 C1 B2 D1 C2 B3 D2 C3 D3
ACT: r(0,0) r(1,0) r(2,0) h3(0) r(3,0) h3(1) h3(2) h3(3)        (sa +1 each)
DVE: r1(0) r1(1) h2(0) r1(2) h2(1) r1(3) out(0) h2(2) out(1)
     h2(3) out(2) out(3)                                         (sv +1 each)
s2 counts PE tail ops (B/C/D) in PE order.
"""

from contextlib import ExitStack

import ml_dtypes
import numpy as np

import concourse.bass as bass
import concourse.mybir as mybir

N_CORES = 8
B = 16384
BC = B // N_CORES
NB = 512
NT = BC // NB
KC = 112
NKC = 7

F32 = mybir.dt.float32
BF16 = mybir.dt.bfloat16
FP16 = mybir.dt.float16
RELU = mybir.ActivationFunctionType.Relu
ADD = mybir.AluOpType.add
MAX = mybir.AluOpType.max

N_WARM_MM = 12

# fused [w1|x] SBUF region offsets (fp16 elements per partition)
OFF_W1A = 0                    # w1 chunks 0:4   -> 4*256 = 1024
OFF_T0A = 1024                 # t0 chunks 0:4   -> 4*512 = 2048
OFF_W1B = 3072                 # w1 chunks 4:7   -> 3*256 = 768
OFF_T0B = 3840                 # t0 chunks 4:7   -> 3*512 = 1536
OFF_T = [5376 + 3584 * i for i in range(3)]   # t1..t3 whole tiles
FUSED_LEN = 5376 + 3584 * 3    # 16128

PE_ORDER = [
    ("A", 0), ("A", 1), ("B", 0), ("A", 2), ("C", 0), ("B", 1), ("A", 3),
    ("D", 0), ("C", 1), ("B", 2), ("D", 1), ("C", 2), ("B", 3), ("D", 2),
    ("C", 3), ("D", 3),
]
TAILS = [(k, t) for (k, t) in PE_ORDER if k != "A"]
POS_PE = {op: i + 1 for i, op in enumerate(TAILS)}  # s2 thresholds

ACT_ORDER = [
    ("r", 0, 0), ("r", 1, 0), ("r", 2, 0), ("h3", 0), ("r", 3, 0),
    ("h3", 1), ("h3", 2), ("h3", 3),
]
POS_A = {op: i + 1 for i, op in enumerate(ACT_ORDER)}  # sa thresholds

DVE_ORDER = [
    ("r1", 0), ("r1", 1), ("h2", 0), ("r1", 2), ("h2", 1), ("r1", 3),
    ("out", 0), ("h2", 2), ("out", 1), ("h2", 3), ("out", 2), ("out", 3),
]
POS_V = {op: i + 1 for i, op in enumerate(DVE_ORDER)}  # sv thresholds


def build_program(l1_dt=FP16, l234_dt=FP16):
    nc = bass.Bass()

    n_wp = 256 + 64 + 10

    # One contiguous DRAM tensor per DMA piece.
    p0a_d = nc.declare_dram_parameter("p0a", [KC, 3072], l1_dt, isOutput=False)
    p0b_d = nc.declare_dram_parameter("p0b", [KC, 2304], l1_dt, isOutput=False)
    pt_d = [
        nc.declare_dram_parameter(f"pt{t}", [KC, NKC * NB], l1_dt, isOutput=False)
        for t in (1, 2, 3)
    ]
    wp_d = nc.declare_dram_parameter("wpack", [128, n_wp], l234_dt, isOutput=False)
    bp_d = nc.declare_dram_parameter("bpack", [128, 5], F32, isOutput=False)
    out_d = nc.declare_dram_parameter("outT", [NT, 10, NB], F32, isOutput=True)

    ctx = ExitStack()
    with ctx:
        fused = ctx.enter_context(nc.sbuf_tensor([KC, FUSED_LEN], l1_dt))
        wpsb = ctx.enter_context(nc.sbuf_tensor([128, n_wp], l234_dt))
        bpsb = ctx.enter_context(nc.sbuf_tensor([128, 5], F32))
        h1sb = ctx.enter_context(nc.sbuf_tensor([128, 2, 2, NB], l234_dt))
        h2sb = ctx.enter_context(nc.sbuf_tensor([128, 2, NB], l234_dt))
        h3sb = ctx.enter_context(nc.sbuf_tensor([64, 2, NB], l234_dt))
        osb = ctx.enter_context(nc.sbuf_tensor([10, NT, NB], F32))
        warm = ctx.enter_context(nc.sbuf_tensor([1, 513], BF16))
        junk = ctx.enter_context(nc.sbuf_tensor([128, 128 + NB], FP16))
        dump_a = ctx.enter_context(nc.sbuf_tensor([1, 16], BF16))

        def w1v(c, m):
            off = (OFF_W1A + c * 256) if c < 4 else (OFF_W1B + (c - 4) * 256)
            off += m * 128
            return fused[:, off : off + 128]

        def xv(t, c):
            if t == 0:
                off = (OFF_T0A + c * 512) if c < 4 else (OFF_T0B + (c - 4) * 512)
            else:
                off = OFF_T[t - 1] + c * 512
            return fused[:, off : off + 512]

        w2v = wpsb[:, 0:256].rearrange("p (c o) -> p c o", c=2)
        w3v = wpsb[:, 256:320]
        w4v = wpsb[0:64, 320:330]
        b1v = bpsb[:, 0:2]
        b2v = bpsb[:, 2:3]
        b3v = bpsb[0:64, 3:4]
        b4v = bpsb[0:10, 4:5]

        ps1 = ctx.enter_context(nc.psum_tensor([128, 2, 2, NB], F32))
        ps2 = ctx.enter_context(nc.psum_tensor([128, NB], F32))
        ps3 = ctx.enter_context(nc.psum_tensor([64, NB], F32))
        ps4 = ctx.enter_context(nc.psum_tensor([10, NB], F32))
        psd = ctx.enter_context(nc.psum_tensor([128, NB], F32))

        # piece sems: sp[0]=p0a, sp[1]=p0b, sp[2..4]=t1..t3
        sp = [ctx.enter_context(nc.semaphore(f"sp{i}")) for i in range(5)]
        swr = ctx.enter_context(nc.semaphore("swr"))
        sm = ctx.enter_context(nc.semaphore("sm"))
        s2 = ctx.enter_context(nc.semaphore("s2"))
        sa = ctx.enter_context(nc.semaphore("sa"))
        sv = ctx.enter_context(nc.semaphore("sv"))
        sof = ctx.enter_context(nc.semaphore("sof"))

        block = ctx.enter_context(nc.Block())

        @block.sync
        def _(sy):
            # qSP ring, need-ordered: wpack, bpack, t1, t3, outs
            sy.dma_start(out=wpsb[:], in_=wp_d[:]).then_inc(swr, 16)
            sy.dma_start(out=bpsb[:], in_=bp_d[:]).then_inc(swr, 16)
            sy.dma_start(
                out=fused[:, OFF_T[0] : OFF_T[0] + 3584], in_=pt_d[0][:]
            ).then_inc(sp[2], 16)
            sy.dma_start(
                out=fused[:, OFF_T[2] : OFF_T[2] + 3584], in_=pt_d[2][:]
            ).then_inc(sp[4], 16)
            for t in range(NT):
                sy.wait_ge(sv, POS_V[("out", t)])
                sy.dma_start(out=out_d[t], in_=osb[:, t, :]).then_inc(sof, 16)
            sy.wait_ge(sof, 16 * NT)

        @block.scalar
        def _(se):
            # qAct ring, need-ordered: [w1a|t0a], [w1b|t0b], t2
            se.dma_start(out=fused[:, 0:3072], in_=p0a_d[:]).then_inc(sp[0], 16)
            se.dma_start(out=fused[:, 3072:5376], in_=p0b_d[:]).then_inc(sp[1], 16)
            se.dma_start(
                out=fused[:, OFF_T[1] : OFF_T[1] + 3584], in_=pt_d[1][:]
            ).then_inc(sp[3], 16)
            se.activation(dump_a[:], warm[:, 0:16], RELU)  # preload relu table
            se.wait_ge(swr, 32)
            for op in ACT_ORDER:
                if op[0] == "r":
                    _, t, _m = op
                    st = t % 2
                    if t >= 2:
                        se.wait_ge(s2, POS_PE[("B", t - 2)])  # h1 slot free
                    se.wait_ge(sm, 2 * t + 1)
                    se.activation(
                        h1sb[:, st, 0, :], ps1[:, st, 0, :], RELU,
                        bias=b1v[:, 0:1],
                    ).then_inc(sa, 1)
                else:
                    _, t = op
                    st = t % 2
                    se.wait_ge(s2, POS_PE[("C", t)])
                    se.activation(
                        h3sb[:, st, :], ps3[:], RELU, bias=b3v[:]
                    ).then_inc(sa, 1)

        @block.vector
        def _(ve):
            ve.wait_ge(swr, 32)
            for op in DVE_ORDER:
                kind, t = op
                st = t % 2
                if kind == "r1":
                    if t >= 2:
                        ve.wait_ge(s2, POS_PE[("B", t - 2)])  # h1 slot free
                    ve.wait_ge(sm, 2 * t + 2)
                    ve.tensor_scalar(
                        h1sb[:, st, 1, :], ps1[:, st, 1, :], b1v[:, 1:2],
                        0.0, ADD, MAX,
                    ).then_inc(sv, 1)
                elif kind == "h2":
                    ve.wait_ge(s2, POS_PE[("B", t)])
                    ve.tensor_scalar(
                        h2sb[:, st, :], ps2[:], b2v[:], 0.0, ADD, MAX
                    ).then_inc(sv, 1)
                else:
                    ve.wait_ge(s2, POS_PE[("D", t)])
                    ve.tensor_scalar(
                        osb[:, t, :], ps4[:], b4v[:], None, ADD
                    ).then_inc(sv, 1)

        @block.tensor
        def _(te):
            # Full-array warmup burst: lifts the HAM clock gate to 8/8 while
            # the first DMAs land. Reads uninitialized SBUF (values
            # irrelevant), dumps into a dedicated psum bank.
            def dummy_mm(k):
                for _i in range(k):
                    te.matmul(psd[:, :], junk[:, 0:128], junk[:, 128:],
                              start=True, stop=True)

            dummy_mm(N_WARM_MM)

            def emit_L1(t):
                st = t % 2
                if t >= 2:
                    te.wait_ge(sa, POS_A[("r", t - 2, 0)])  # ps1 m0 free
                    te.wait_ge(sv, POS_V[("r1", t - 2)])    # ps1 m1 free
                for c in range(NKC):
                    if t == 0:
                        if c == 0:
                            te.wait_ge(sp[0], 16)
                        elif c == 4:
                            te.wait_ge(sp[1], 16)
                    elif c == 0:
                        te.wait_ge(sp[1], 16)      # w1b lives in piece 1
                        te.wait_ge(sp[t + 1], 16)
                    for m in range(2):
                        mm = te.matmul(
                            ps1[:, st, m, :],
                            w1v(c, m),
                            xv(t, c),
                            start=(c == 0),
                            stop=(c == NKC - 1),
                        )
                        if c == NKC - 1:
                            mm.then_inc(sm, 1)

            for kind, t in PE_ORDER:
                st = t % 2
                if kind == "A":
                    if t >= 1:
                        dummy_mm(2 if t < 3 else 3)  # warmth insurance
                    emit_L1(t)
                elif kind == "B":
                    if t == 0:
                        te.wait_ge(swr, 32)
                    te.wait_ge(sa, POS_A[("r", t, 0)])
                    if t >= 1:
                        te.wait_ge(sv, POS_V[("h2", t - 1)])  # ps2 free
                    te.matmul(
                        ps2[:], w2v[:, 0, :], h1sb[:, st, 0, :],
                        start=True, stop=False,
                    )
                    te.wait_ge(sv, POS_V[("r1", t)])
                    te.matmul(
                        ps2[:], w2v[:, 1, :], h1sb[:, st, 1, :],
                        start=False, stop=True,
                    ).then_inc(s2, 1)
                elif kind == "C":
                    if t == 3:
                        dummy_mm(1)
                    te.wait_ge(sv, POS_V[("h2", t)])
                    te.matmul(
                        ps3[:], w3v[:], h2sb[:, st, :], start=True, stop=True
                    ).then_inc(s2, 1)
                else:
                    if t == 3:
                        dummy_mm(2)
                    te.wait_ge(sa, POS_A[("h3", t)])
                    if t >= 1:
                        te.wait_ge(sv, POS_V[("out", t - 1)])  # ps4 free
                    te.matmul(
                        ps4[:], w4v[:], h3sb[:, st, :], start=True, stop=True
                    ).then_inc(s2, 1)

    return nc


def _np_dt(dt):
    if dt == BF16:
        return ml_dtypes.bfloat16
    if dt == FP16:
        return np.float16
    return np.float32


def prepare_inputs(x, conv_w, w1, b1, w2, b2, w3, b3, w4, b4,
                   l1_dt=FP16, l234_dt=FP16):
    w1v = np.ascontiguousarray(w1.T).reshape(26, 26, 256)
    w1e = np.zeros((28, 28, 256), dtype=np.float32)
    for di in range(3):
        for dj in range(3):
            w1e[di : di + 26, dj : dj + 26, :] += conv_w[di, dj] * w1v
    w1e = w1e.reshape(784, 256)
    # [KC, NKC, 256] chunk-major per partition
    w1t = np.ascontiguousarray(
        w1e.reshape(NKC, KC, 256).transpose(1, 0, 2)
    ).astype(_np_dt(l1_dt))

    w2t = np.ascontiguousarray(w2.T).reshape(2, 128, 128).transpose(1, 0, 2)
    wpack = np.zeros((128, 256 + 64 + 10), dtype=np.float32)
    wpack[:, 0:256] = w2t.reshape(128, 256)
    wpack[:, 256:320] = w3.T
    wpack[0:64, 320:330] = w4.T
    wpack = wpack.astype(_np_dt(l234_dt))

    bpack = np.zeros((128, 5), dtype=np.float32)
    bpack[:, 0:2] = b1.reshape(2, 128).T
    bpack[:, 2] = b2
    bpack[0:64, 3] = b3
    bpack[0:10, 4] = b4

    shared = {"wpack": wpack, "bpack": bpack}
    in_maps = []
    for m in range(N_CORES):
        xc = x[m * BC : (m + 1) * BC]
        # [NT, KC, NKC, NB] pixels-on-partitions, chunk-major free dim
        xt = np.ascontiguousarray(
            xc.reshape(NT, NB, NKC, KC).transpose(0, 3, 2, 1)
        ).astype(_np_dt(l1_dt))
        d = dict(shared)
        # fused pieces: [w1 chunks | t0 chunks] halves, then whole tiles
        d["p0a"] = np.ascontiguousarray(
            np.concatenate(
                [w1t[:, 0:4, :].reshape(KC, 1024), xt[0, :, 0:4, :].reshape(KC, 2048)],
                axis=1,
            )
        )
        d["p0b"] = np.ascontiguousarray(
            np.concatenate(
                [w1t[:, 4:7, :].reshape(KC, 768), xt[0, :, 4:7, :].reshape(KC, 1536)],
                axis=1,
            )
        )
        for t in (1, 2, 3):
            d[f"pt{t}"] = np.ascontiguousarray(xt[t].reshape(KC, NKC * NB))
        in_maps.append(d)
    return in_maps



_PROGRAM = None


def _get_program():
    global _PROGRAM
    if _PROGRAM is None:
        _PROGRAM = build_program()
    return _PROGRAM


def kernel(x, conv_w, w1, b1, w2, b2, w3, b3, w4, b4):
    from concourse import bass_utils

    args = [x, conv_w, w1, b1, w2, b2, w3, b3, w4, b4]
    x, conv_w, w1, b1, w2, b2, w3, b3, w4, b4 = [
        np.asarray(a, dtype=np.float32) for a in args
    ]
    nc = _get_program()
    in_maps = prepare_inputs(x, conv_w, w1, b1, w2, b2, w3, b3, w4, b4)
    res = bass_utils.run_bass_kernel_spmd(nc, in_maps, list(range(N_CORES)))
    out = np.concatenate(
        [
            res.results[m]["outT"].transpose(0, 2, 1).reshape(BC, 10)
            for m in range(N_CORES)
        ],
        axis=0,
    )
    return out.astype(np.float32)


# revision 13
# speedup vs baseline: 1.0741x; 1.0741x over previous
"""Trainium2 Bass kernel for nn_DigitConvolutionalModel (dense CNN -> MLP).

Pure data parallel over 8 NeuronCores (2048 samples each). The 3x3 conv is
linear, so the host folds it into the first FC layer (W1e = C @ w1.T), making
the whole network a 4-layer MLP computed in transposed orientation (features
on partitions, batch on the free dim) in fp16 (psum fp32, ~5e-4 rel err):

    outT = w4t.T @ relu(w3t.T @ relu(w2t.T @ relu(W1e.T @ xT + b1) + b2) + b3) + b4

Raw bass with manual semaphores. DMA pieces are contiguous DRAM tensors
spread across both HWDGE rings in strict need order with balanced bytes
(per-ring throughput is ~135 GB/s; aggregate ~260). The tensor engine opens
with a full-array warmup burst sized to the DMA-bound L1 start (~12.5us) so
the HAM clock-gate reaches 8/8 before real work, and dummy matmuls bridge
the x-tile waits so the PE never re-throttles. The two h1-relu halves run in
parallel on ACT (m0) and DVE (m1).

PE op order (A=L1, B=L2, C=L3, D=L4):
  A0 A1 B0 A2 C0 B1 A3 D0 C1 B2 D1 C2 B3 D2 C3 D3
ACT: r(0,0) r(1,0) r(2,0) h3(0) r(3,0) h3(1) h3(2) h3(3)        (sa +1 each)
DVE: r1(0) r1(1) h2(0) r1(2) h2(1) r1(3) out(0) h2(2) out(1)
     h2(3) out(2) out(3)                                         (sv +1 each)
s2 counts PE tail ops (B/C/D) in PE order.
"""

from contextlib import ExitStack

import ml_dtypes
import numpy as np

import concourse.bass as bass
import concourse.mybir as mybir

N_CORES = 8
B = 16384
BC = B // N_CORES
NB = 512
NT = BC // NB
KC = 112
NKC = 7

F32 = mybir.dt.float32
BF16 = mybir.dt.bfloat16
FP16 = mybir.dt.float16
RELU = mybir.ActivationFunctionType.Relu
ADD = mybir.AluOpType.add
MAX = mybir.AluOpType.max

N_WARM_MM = 13

# t0 in halves for an earlier L1 start; t1-t3 whole (fewer receipt stalls)
X_SPLITS = [[(0, 4), (4, 7)], [(0, 7)], [(0, 7)], [(0, 7)]]
W1_SPLITS = [(0, 7)]

PE_ORDER = [
    ("A", 0), ("A", 1), ("B", 0), ("A", 2), ("C", 0), ("B", 1), ("A", 3),
    ("D", 0), ("C", 1), ("B", 2), ("D", 1), ("C", 2), ("B", 3), ("D", 2),
    ("C", 3), ("D", 3),
]
TAILS = [(k, t) for (k, t) in PE_ORDER if k != "A"]
POS_PE = {op: i + 1 for i, op in enumerate(TAILS)}  # s2 thresholds

ACT_ORDER = [
    ("r", 0, 0), ("r", 1, 0), ("r", 2, 0), ("h3", 0), ("r", 3, 0),
    ("h3", 1), ("h3", 2), ("h3", 3),
]
POS_A = {op: i + 1 for i, op in enumerate(ACT_ORDER)}  # sa thresholds

DVE_ORDER = [
    ("r1", 0), ("r1", 1), ("h2", 0), ("r1", 2), ("h2", 1), ("r1", 3),
    ("out", 0), ("h2", 2), ("out", 1), ("h2", 3), ("out", 2), ("out", 3),
]
POS_V = {op: i + 1 for i, op in enumerate(DVE_ORDER)}  # sv thresholds


def build_program(l1_dt=FP16, l234_dt=FP16):
    nc = bass.Bass()

    n_wp = 256 + 64 + 10

    # One contiguous DRAM tensor per DMA piece.
    xp_d = [
        [
            nc.declare_dram_parameter(
                f"xp{t}_{i}", [KC, (c1 - c0) * NB], l1_dt, isOutput=False
            )
            for i, (c0, c1) in enumerate(X_SPLITS[t])
        ]
        for t in range(NT)
    ]
    w1p_d = [
        nc.declare_dram_parameter(
            f"w1p{i}", [KC, (c1 - c0) * 256], l1_dt, isOutput=False
        )
        for i, (c0, c1) in enumerate(W1_SPLITS)
    ]
    wp_d = nc.declare_dram_parameter("wpack", [128, n_wp], l234_dt, isOutput=False)
    bp_d = nc.declare_dram_parameter("bpack", [128, 5], F32, isOutput=False)
    out_d = nc.declare_dram_parameter("outT", [NT, 10, NB], F32, isOutput=True)

    ctx = ExitStack()
    with ctx:
        xsb = ctx.enter_context(nc.sbuf_tensor([KC, NT, NKC, NB], l1_dt))
        w1sb = ctx.enter_context(nc.sbuf_tensor([KC, NKC, 256], l1_dt))
        wpsb = ctx.enter_context(nc.sbuf_tensor([128, n_wp], l234_dt))
        bpsb = ctx.enter_context(nc.sbuf_tensor([128, 5], F32))
        h1sb = ctx.enter_context(nc.sbuf_tensor([128, 2, 2, NB], l234_dt))
        h2sb = ctx.enter_context(nc.sbuf_tensor([128, 2, NB], l234_dt))
        h3sb = ctx.enter_context(nc.sbuf_tensor([64, 2, NB], l234_dt))
        osb = ctx.enter_context(nc.sbuf_tensor([10, NT, NB], F32))
        warm = ctx.enter_context(nc.sbuf_tensor([1, 513], BF16))
        junk = ctx.enter_context(nc.sbuf_tensor([128, 128 + NB], FP16))
        dump_a = ctx.enter_context(nc.sbuf_tensor([1, 16], BF16))

        w2v = wpsb[:, 0:256].rearrange("p (c o) -> p c o", c=2)
        w3v = wpsb[:, 256:320]
        w4v = wpsb[0:64, 320:330]
        b1v = bpsb[:, 0:2]
        b2v = bpsb[:, 2:3]
        b3v = bpsb[0:64, 3:4]
        b4v = bpsb[0:10, 4:5]

        ps1 = ctx.enter_context(nc.psum_tensor([128, 2, 2, NB], F32))
        ps2 = ctx.enter_context(nc.psum_tensor([128, NB], F32))
        ps3 = ctx.enter_context(nc.psum_tensor([64, NB], F32))
        ps4 = ctx.enter_context(nc.psum_tensor([10, NB], F32))
        psd = ctx.enter_context(nc.psum_tensor([128, NB], F32))

        sx = [
            [ctx.enter_context(nc.semaphore(f"sx{t}_{i}")) for i in range(len(X_SPLITS[t]))]
            for t in range(NT)
        ]
        sw1 = [ctx.enter_context(nc.semaphore(f"sw1_{i}")) for i in range(len(W1_SPLITS))]
        swr = ctx.enter_context(nc.semaphore("swr"))
        sm = ctx.enter_context(nc.semaphore("sm"))
        s2 = ctx.enter_context(nc.semaphore("s2"))
        sa = ctx.enter_context(nc.semaphore("sa"))
        sv = ctx.enter_context(nc.semaphore("sv"))
        sof = ctx.enter_context(nc.semaphore("sof"))

        block = ctx.enter_context(nc.Block())

        @block.sync
        def _(sy):
            # qSP ring, need-ordered: w1, wpack, bpack, t2, t3, outs
            sy.dma_start(out=w1sb[:, :, :], in_=w1p_d[0][:]).then_inc(sw1[0], 16)
            sy.dma_start(out=wpsb[:], in_=wp_d[:]).then_inc(swr, 16)
            sy.dma_start(out=bpsb[:], in_=bp_d[:]).then_inc(swr, 16)
            sy.dma_start(out=xsb[:, 2, :, :], in_=xp_d[2][0][:]).then_inc(
                sx[2][0], 16
            )
            sy.dma_start(out=xsb[:, 3, :, :], in_=xp_d[3][0][:]).then_inc(
                sx[3][0], 16
            )
            for t in range(NT):
                sy.wait_ge(sv, POS_V[("out", t)])
                sy.dma_start(out=out_d[t], in_=osb[:, t, :]).then_inc(sof, 16)
            sy.wait_ge(sof, 16 * NT)

        @block.scalar
        def _(se):
            # qAct ring, need-ordered: t0A, t0B, t1
            se.dma_start(out=xsb[:, 0, 0:4, :], in_=xp_d[0][0][:]).then_inc(
                sx[0][0], 16
            )
            se.dma_start(out=xsb[:, 0, 4:7, :], in_=xp_d[0][1][:]).then_inc(
                sx[0][1], 16
            )
            se.dma_start(out=xsb[:, 1, :, :], in_=xp_d[1][0][:]).then_inc(
                sx[1][0], 16
            )
            se.activation(dump_a[:], warm[:, 0:16], RELU)  # preload relu table
            se.wait_ge(swr, 32)
            for op in ACT_ORDER:
                if op[0] == "r":
                    _, t, _m = op
                    st = t % 2
                    if t >= 2:
                        se.wait_ge(s2, POS_PE[("B", t - 2)])  # h1 slot free
                    se.wait_ge(sm, 2 * t + 1)
                    se.activation(
                        h1sb[:, st, 0, :], ps1[:, st, 0, :], RELU,
                        bias=b1v[:, 0:1],
                    ).then_inc(sa, 1)
                else:
                    _, t = op
                    st = t % 2
                    se.wait_ge(s2, POS_PE[("C", t)])
                    se.activation(
                        h3sb[:, st, :], ps3[:], RELU, bias=b3v[:]
                    ).then_inc(sa, 1)

        @block.vector
        def _(ve):
            ve.wait_ge(swr, 32)
            for op in DVE_ORDER:
                kind, t = op
                st = t % 2
                if kind == "r1":
                    if t >= 2:
                        ve.wait_ge(s2, POS_PE[("B", t - 2)])  # h1 slot free
                    ve.wait_ge(sm, 2 * t + 2)
                    ve.tensor_scalar(
                        h1sb[:, st, 1, :], ps1[:, st, 1, :], b1v[:, 1:2],
                        0.0, ADD, MAX,
                    ).then_inc(sv, 1)
                elif kind == "h2":
                    ve.wait_ge(s2, POS_PE[("B", t)])
                    ve.tensor_scalar(
                        h2sb[:, st, :], ps2[:], b2v[:], 0.0, ADD, MAX
                    ).then_inc(sv, 1)
                else:
                    ve.wait_ge(s2, POS_PE[("D", t)])
                    ve.tensor_scalar(
                        osb[:, t, :], ps4[:], b4v[:], None, ADD
                    ).then_inc(sv, 1)

        @block.tensor
        def _(te):
            # Full-array warmup burst: lifts the HAM clock gate to 8/8 while
            # the first DMAs land. Reads uninitialized SBUF (values
            # irrelevant), dumps into a dedicated psum bank.
            def dummy_mm(k):
                for _i in range(k):
                    te.matmul(psd[:, :], junk[:, 0:128], junk[:, 128:],
                              start=True, stop=True)

            dummy_mm(N_WARM_MM)

            def emit_L1(t):
                st = t % 2
                if t >= 2:
                    te.wait_ge(sa, POS_A[("r", t - 2, 0)])  # ps1 m0 free
                    te.wait_ge(sv, POS_V[("r1", t - 2)])    # ps1 m1 free
                for c in range(NKC):
                    for i, (a, _b) in enumerate(X_SPLITS[t]):
                        if a == c:
                            te.wait_ge(sx[t][i], 16)
                    if t == 0:
                        for i, (a, _b) in enumerate(W1_SPLITS):
                            if a == c:
                                te.wait_ge(sw1[i], 16)
                    for m in range(2):
                        mm = te.matmul(
                            ps1[:, st, m, :],
                            w1sb[:, c, m * 128 : (m + 1) * 128],
                            xsb[:, t, c, :],
                            start=(c == 0),
                            stop=(c == NKC - 1),
                        )
                        if c == NKC - 1:
                            mm.then_inc(sm, 1)

            for kind, t in PE_ORDER:
                st = t % 2
                if kind == "A":
                    if t >= 1:
                        dummy_mm(2 if t < 3 else 3)  # warmth insurance
                    emit_L1(t)
                elif kind == "B":
                    if t == 0:
                        te.wait_ge(swr, 32)
                    te.wait_ge(sa, POS_A[("r", t, 0)])
                    if t >= 1:
                        te.wait_ge(sv, POS_V[("h2", t - 1)])  # ps2 free
                    te.matmul(
                        ps2[:], w2v[:, 0, :], h1sb[:, st, 0, :],
                        start=True, stop=False,
                    )
                    te.wait_ge(sv, POS_V[("r1", t)])
                    te.matmul(
                        ps2[:], w2v[:, 1, :], h1sb[:, st, 1, :],
                        start=False, stop=True,
                    ).then_inc(s2, 1)
                elif kind == "C":
                    if t == 3:
                        dummy_mm(1)
                    te.wait_ge(sv, POS_V[("h2", t)])
                    te.matmul(
                        ps3[:], w3v[:], h2sb[:, st, :], start=True, stop=True
                    ).then_inc(s2, 1)
                else:
                    if t == 3:
                        dummy_mm(2)
                    te.wait_ge(sa, POS_A[("h3", t)])
                    if t >= 1:
                        te.wait_ge(sv, POS_V[("out", t - 1)])  # ps4 free
                    te.matmul(
                        ps4[:], w4v[:], h3sb[:, st, :], start=True, stop=True
                    ).then_inc(s2, 1)

    return nc


def _np_dt(dt):
    if dt == BF16:
        return ml_dtypes.bfloat16
    if dt == FP16:
        return np.float16
    return np.float32


def prepare_inputs(x, conv_w, w1, b1, w2, b2, w3, b3, w4, b4,
                   l1_dt=FP16, l234_dt=FP16):
    w1v = np.ascontiguousarray(w1.T).reshape(26, 26, 256)
    w1e = np.zeros((28, 28, 256), dtype=np.float32)
    for di in range(3):
        for dj in range(3):
            w1e[di : di + 26, dj : dj + 26, :] += conv_w[di, dj] * w1v
    w1e = w1e.reshape(784, 256)
    w1t = np.ascontiguousarray(
        w1e.reshape(NKC, KC, 256).transpose(1, 0, 2)
    ).reshape(KC, NKC * 256).astype(_np_dt(l1_dt))
    w1pieces = {}
    for i, (c0, c1) in enumerate(W1_SPLITS):
        w1pieces[f"w1p{i}"] = np.ascontiguousarray(
            w1t.reshape(KC, NKC, 256)[:, c0:c1, :].reshape(KC, (c1 - c0) * 256)
        )

    w2t = np.ascontiguousarray(w2.T).reshape(2, 128, 128).transpose(1, 0, 2)
    wpack = np.zeros((128, 256 + 64 + 10), dtype=np.float32)
    wpack[:, 0:256] = w2t.reshape(128, 256)
    wpack[:, 256:320] = w3.T
    wpack[0:64, 320:330] = w4.T
    wpack = wpack.astype(_np_dt(l234_dt))

    bpack = np.zeros((128, 5), dtype=np.float32)
    bpack[:, 0:2] = b1.reshape(2, 128).T
    bpack[:, 2] = b2
    bpack[0:64, 3] = b3
    bpack[0:10, 4] = b4

    shared = {"wpack": wpack, "bpack": bpack, **w1pieces}
    in_maps = []
    for m in range(N_CORES):
        xc = x[m * BC : (m + 1) * BC]
        xt = np.ascontiguousarray(
            xc.reshape(NT, NB, NKC, KC).transpose(0, 3, 2, 1)
        ).astype(_np_dt(l1_dt))
        d = dict(shared)
        for t in range(NT):
            for i, (c0, c1) in enumerate(X_SPLITS[t]):
                d[f"xp{t}_{i}"] = np.ascontiguousarray(
                    xt[t, :, c0:c1, :].reshape(KC, (c1 - c0) * NB)
                )
        in_maps.append(d)
    return in_maps



_PROGRAM = None


def _get_program():
    global _PROGRAM
    if _PROGRAM is None:
        _PROGRAM = build_program()
    return _PROGRAM


def kernel(x, conv_w, w1, b1, w2, b2, w3, b3, w4, b4):
    from concourse import bass_utils

    args = [x, conv_w, w1, b1, w2, b2, w3, b3, w4, b4]
    x, conv_w, w1, b1, w2, b2, w3, b3, w4, b4 = [
        np.asarray(a, dtype=np.float32) for a in args
    ]
    nc = _get_program()
    in_maps = prepare_inputs(x, conv_w, w1, b1, w2, b2, w3, b3, w4, b4)
    res = bass_utils.run_bass_kernel_spmd(nc, in_maps, list(range(N_CORES)))
    out = np.concatenate(
        [
            res.results[m]["outT"].transpose(0, 2, 1).reshape(BC, 10)
            for m in range(N_CORES)
        ],
        axis=0,
    )
    return out.astype(np.float32)


# revision 14
# speedup vs baseline: 1.1231x; 1.0456x over previous
"""Trainium2 Bass kernel for nn_DigitConvolutionalModel (dense CNN -> MLP).

Pure data parallel over 8 NeuronCores (2048 samples each). The 3x3 conv is
linear, so the host folds it into the first FC layer (W1e = C @ w1.T), making
the whole network a 4-layer MLP computed in transposed orientation (features
on partitions, batch on the free dim) in fp16 (psum fp32, ~5e-4 rel err):

    outT = w4t.T @ relu(w3t.T @ relu(w2t.T @ relu(W1e.T @ xT + b1) + b2) + b3) + b4

Raw bass with manual semaphores. DMA pieces are contiguous DRAM tensors
spread across both HWDGE rings in strict need order with balanced bytes
(per-ring throughput is ~135 GB/s; aggregate ~260). The tensor engine opens
with a full-array warmup burst sized to the DMA-bound L1 start (~12.5us) so
the HAM clock-gate reaches 8/8 before real work, and dummy matmuls bridge
the x-tile waits so the PE never re-throttles. The two h1-relu halves run in
parallel on ACT (m0) and DVE (m1).

PE op order (A=L1, B=L2, C=L3, D=L4):
  A0 A1 B0 A2 C0 B1 A3 D0 C1 B2 D1 C2 B3 D2 C3 D3
ACT: r(0,0) r(1,0) r(2,0) h3(0) r(3,0) h3(1) h3(2) h3(3)        (sa +1 each)
DVE: r1(0) r1(1) h2(0) r1(2) h2(1) r1(3) out(0) h2(2) out(1)
     h2(3) out(2) out(3)                                         (sv +1 each)
s2 counts PE tail ops (B/C/D) in PE order.
"""

from contextlib import ExitStack

import ml_dtypes
import numpy as np

import concourse.bass as bass
import concourse.mybir as mybir

N_CORES = 8
B = 16384
BC = B // N_CORES
NB = 512
NT = BC // NB
KC = 112
NKC = 7

F32 = mybir.dt.float32
BF16 = mybir.dt.bfloat16
FP16 = mybir.dt.float16
RELU = mybir.ActivationFunctionType.Relu
ADD = mybir.AluOpType.add
MAX = mybir.AluOpType.max

N_WARM_MM = 15

# t0 in halves for an earlier L1 start; t1-t3 whole (fewer receipt stalls)
X_SPLITS = [[(0, 4), (4, 7)], [(0, 7)], [(0, 7)], [(0, 7)]]
W1_SPLITS = [(0, 7)]

PE_ORDER = [
    ("A", 0), ("A", 1), ("B", 0), ("A", 2), ("C", 0), ("B", 1), ("A", 3),
    ("D", 0), ("C", 1), ("B", 2), ("D", 1), ("C", 2), ("B", 3), ("D", 2),
    ("C", 3), ("D", 3),
]
TAILS = [(k, t) for (k, t) in PE_ORDER if k != "A"]
POS_PE = {op: i + 1 for i, op in enumerate(TAILS)}  # s2 thresholds

ACT_ORDER = [
    ("r", 0, 0), ("r", 1, 0), ("r", 2, 0), ("h3", 0), ("r", 3, 0),
    ("h3", 1), ("h3", 2), ("h3", 3),
]
POS_A = {op: i + 1 for i, op in enumerate(ACT_ORDER)}  # sa thresholds

DVE_ORDER = [
    ("r1", 0), ("r1", 1), ("h2", 0), ("r1", 2), ("h2", 1), ("r1", 3),
    ("out", 0), ("h2", 2), ("out", 1), ("h2", 3), ("out", 2), ("out", 3),
]
POS_V = {op: i + 1 for i, op in enumerate(DVE_ORDER)}  # sv thresholds


def build_program(l1_dt=FP16, l234_dt=FP16):
    nc = bass.Bass()

    n_wp = 256 + 64 + 10

    # One contiguous DRAM tensor per DMA piece.
    xp_d = [
        [
            nc.declare_dram_parameter(
                f"xp{t}_{i}", [KC, (c1 - c0) * NB], l1_dt, isOutput=False
            )
            for i, (c0, c1) in enumerate(X_SPLITS[t])
        ]
        for t in range(NT)
    ]
    w1p_d = [
        nc.declare_dram_parameter(
            f"w1p{i}", [KC, (c1 - c0) * 256], l1_dt, isOutput=False
        )
        for i, (c0, c1) in enumerate(W1_SPLITS)
    ]
    wp_d = nc.declare_dram_parameter("wpack", [128, n_wp], l234_dt, isOutput=False)
    bp_d = nc.declare_dram_parameter("bpack", [128, 5], F32, isOutput=False)
    out_d = nc.declare_dram_parameter("outT", [NT, 10, NB], F32, isOutput=True)

    ctx = ExitStack()
    with ctx:
        xsb = ctx.enter_context(nc.sbuf_tensor([KC, NT, NKC, NB], l1_dt))
        w1sb = ctx.enter_context(nc.sbuf_tensor([KC, NKC, 256], l1_dt))
        wpsb = ctx.enter_context(nc.sbuf_tensor([128, n_wp], l234_dt))
        bpsb = ctx.enter_context(nc.sbuf_tensor([128, 5], F32))
        h1sb = ctx.enter_context(nc.sbuf_tensor([128, 2, 2, NB], l234_dt))
        h2sb = ctx.enter_context(nc.sbuf_tensor([128, 2, NB], l234_dt))
        h3sb = ctx.enter_context(nc.sbuf_tensor([64, 2, NB], l234_dt))
        osb = ctx.enter_context(nc.sbuf_tensor([10, NT, NB], F32))
        warm = ctx.enter_context(nc.sbuf_tensor([1, 513], BF16))
        junk = ctx.enter_context(nc.sbuf_tensor([128, 128 + NB], FP16))
        dump_a = ctx.enter_context(nc.sbuf_tensor([1, 16], BF16))

        w2v = wpsb[:, 0:256].rearrange("p (c o) -> p c o", c=2)
        w3v = wpsb[:, 256:320]
        w4v = wpsb[0:64, 320:330]
        b1v = bpsb[:, 0:2]
        b2v = bpsb[:, 2:3]
        b3v = bpsb[0:64, 3:4]
        b4v = bpsb[0:10, 4:5]

        ps1 = ctx.enter_context(nc.psum_tensor([128, 2, 2, NB], F32))
        ps2 = ctx.enter_context(nc.psum_tensor([128, NB], F32))
        ps3 = ctx.enter_context(nc.psum_tensor([64, NB], F32))
        ps4 = ctx.enter_context(nc.psum_tensor([10, NB], F32))
        psd = ctx.enter_context(nc.psum_tensor([128, NB], F32))

        sx = [
            [ctx.enter_context(nc.semaphore(f"sx{t}_{i}")) for i in range(len(X_SPLITS[t]))]
            for t in range(NT)
        ]
        sw1 = [ctx.enter_context(nc.semaphore(f"sw1_{i}")) for i in range(len(W1_SPLITS))]
        swr = ctx.enter_context(nc.semaphore("swr"))
        sm = ctx.enter_context(nc.semaphore("sm"))
        s2 = ctx.enter_context(nc.semaphore("s2"))
        sa = ctx.enter_context(nc.semaphore("sa"))
        sv = ctx.enter_context(nc.semaphore("sv"))
        sof = ctx.enter_context(nc.semaphore("sof"))

        block = ctx.enter_context(nc.Block())

        @block.sync
        def _(sy):
            # qSP ring, need-ordered: w1, wpack, bpack, t2, t3, outs
            sy.dma_start(out=w1sb[:, :, :], in_=w1p_d[0][:]).then_inc(sw1[0], 16)
            sy.dma_start(out=wpsb[:], in_=wp_d[:]).then_inc(swr, 16)
            sy.dma_start(out=bpsb[:], in_=bp_d[:]).then_inc(swr, 16)
            sy.dma_start(out=xsb[:, 2, :, :], in_=xp_d[2][0][:]).then_inc(
                sx[2][0], 16
            )
            sy.dma_start(out=xsb[:, 3, :, :], in_=xp_d[3][0][:]).then_inc(
                sx[3][0], 16
            )
            for t in range(NT):
                sy.wait_ge(sv, POS_V[("out", t)])
                sy.dma_start(out=out_d[t], in_=osb[:, t, :]).then_inc(sof, 16)
            sy.wait_ge(sof, 16 * NT)

        @block.scalar
        def _(se):
            # qAct ring, need-ordered: t0A, t0B, t1
            se.dma_start(out=xsb[:, 0, 0:4, :], in_=xp_d[0][0][:]).then_inc(
                sx[0][0], 16
            )
            se.dma_start(out=xsb[:, 0, 4:7, :], in_=xp_d[0][1][:]).then_inc(
                sx[0][1], 16
            )
            se.dma_start(out=xsb[:, 1, :, :], in_=xp_d[1][0][:]).then_inc(
                sx[1][0], 16
            )
            se.activation(dump_a[:], warm[:, 0:16], RELU)  # preload relu table
            se.wait_ge(swr, 32)
            for op in ACT_ORDER:
                if op[0] == "r":
                    _, t, _m = op
                    st = t % 2
                    if t >= 2:
                        se.wait_ge(s2, POS_PE[("B", t - 2)])  # h1 slot free
                    se.wait_ge(sm, 2 * t + 1)
                    se.activation(
                        h1sb[:, st, 0, :], ps1[:, st, 0, :], RELU,
                        bias=b1v[:, 0:1],
                    ).then_inc(sa, 1)
                else:
                    _, t = op
                    st = t % 2
                    se.wait_ge(s2, POS_PE[("C", t)])
                    se.activation(
                        h3sb[:, st, :], ps3[:], RELU, bias=b3v[:]
                    ).then_inc(sa, 1)

        @block.vector
        def _(ve):
            ve.wait_ge(swr, 32)
            for op in DVE_ORDER:
                kind, t = op
                st = t % 2
                if kind == "r1":
                    if t >= 2:
                        ve.wait_ge(s2, POS_PE[("B", t - 2)])  # h1 slot free
                    ve.wait_ge(sm, 2 * t + 2)
                    ve.tensor_scalar(
                        h1sb[:, st, 1, :], ps1[:, st, 1, :], b1v[:, 1:2],
                        0.0, ADD, MAX,
                    ).then_inc(sv, 1)
                elif kind == "h2":
                    ve.wait_ge(s2, POS_PE[("B", t)])
                    ve.tensor_scalar(
                        h2sb[:, st, :], ps2[:], b2v[:], 0.0, ADD, MAX
                    ).then_inc(sv, 1)
                else:
                    ve.wait_ge(s2, POS_PE[("D", t)])
                    ve.tensor_scalar(
                        osb[:, t, :], ps4[:], b4v[:], None, ADD
                    ).then_inc(sv, 1)

        @block.tensor
        def _(te):
            # Full-array warmup burst: lifts the HAM clock gate to 8/8 while
            # the first DMAs land. Reads uninitialized SBUF (values
            # irrelevant), dumps into a dedicated psum bank.
            def dummy_mm(k):
                for _i in range(k):
                    te.matmul(psd[:, :], junk[:, 0:128], junk[:, 128:],
                              start=True, stop=True)

            dummy_mm(N_WARM_MM)

            def emit_L1(t):
                st = t % 2
                if t >= 2:
                    te.wait_ge(sa, POS_A[("r", t - 2, 0)])  # ps1 m0 free
                    te.wait_ge(sv, POS_V[("r1", t - 2)])    # ps1 m1 free
                for c in range(NKC):
                    for i, (a, _b) in enumerate(X_SPLITS[t]):
                        if a == c:
                            te.wait_ge(sx[t][i], 16)
                    if t == 0:
                        for i, (a, _b) in enumerate(W1_SPLITS):
                            if a == c:
                                te.wait_ge(sw1[i], 16)
                    for m in range(2):
                        mm = te.matmul(
                            ps1[:, st, m, :],
                            w1sb[:, c, m * 128 : (m + 1) * 128],
                            xsb[:, t, c, :],
                            start=(c == 0),
                            stop=(c == NKC - 1),
                        )
                        if c == NKC - 1:
                            mm.then_inc(sm, 1)

            for kind, t in PE_ORDER:
                st = t % 2
                if kind == "A":
                    if t >= 1:
                        dummy_mm(3)  # warmth insurance
                    emit_L1(t)
                elif kind == "B":
                    if t == 0:
                        te.wait_ge(swr, 32)
                    te.wait_ge(sa, POS_A[("r", t, 0)])
                    if t >= 1:
                        te.wait_ge(sv, POS_V[("h2", t - 1)])  # ps2 free
                    te.matmul(
                        ps2[:], w2v[:, 0, :], h1sb[:, st, 0, :],
                        start=True, stop=False,
                    )
                    te.wait_ge(sv, POS_V[("r1", t)])
                    te.matmul(
                        ps2[:], w2v[:, 1, :], h1sb[:, st, 1, :],
                        start=False, stop=True,
                    ).then_inc(s2, 1)
                elif kind == "C":
                    if t == 3:
                        dummy_mm(1)
                    te.wait_ge(sv, POS_V[("h2", t)])
                    te.matmul(
                        ps3[:], w3v[:], h2sb[:, st, :], start=True, stop=True
                    ).then_inc(s2, 1)
                else:
                    if t == 3:
                        dummy_mm(2)
                    te.wait_ge(sa, POS_A[("h3", t)])
                    if t >= 1:
                        te.wait_ge(sv, POS_V[("out", t - 1)])  # ps4 free
                    te.matmul(
                        ps4[:], w4v[:], h3sb[:, st, :], start=True, stop=True
                    ).then_inc(s2, 1)

    return nc


def _np_dt(dt):
    if dt == BF16:
        return ml_dtypes.bfloat16
    if dt == FP16:
        return np.float16
    return np.float32


def prepare_inputs(x, conv_w, w1, b1, w2, b2, w3, b3, w4, b4,
                   l1_dt=FP16, l234_dt=FP16):
    w1v = np.ascontiguousarray(w1.T).reshape(26, 26, 256)
    w1e = np.zeros((28, 28, 256), dtype=np.float32)
    for di in range(3):
        for dj in range(3):
            w1e[di : di + 26, dj : dj + 26, :] += conv_w[di, dj] * w1v
    w1e = w1e.reshape(784, 256)
    w1t = np.ascontiguousarray(
        w1e.reshape(NKC, KC, 256).transpose(1, 0, 2)
    ).reshape(KC, NKC * 256).astype(_np_dt(l1_dt))
    w1pieces = {}
    for i, (c0, c1) in enumerate(W1_SPLITS):
        w1pieces[f"w1p{i}"] = np.ascontiguousarray(
            w1t.reshape(KC, NKC, 256)[:, c0:c1, :].reshape(KC, (c1 - c0) * 256)
        )

    w2t = np.ascontiguousarray(w2.T).reshape(2, 128, 128).transpose(1, 0, 2)
    wpack = np.zeros((128, 256 + 64 + 10), dtype=np.float32)
    wpack[:, 0:256] = w2t.reshape(128, 256)
    wpack[:, 256:320] = w3.T
    wpack[0:64, 320:330] = w4.T
    wpack = wpack.astype(_np_dt(l234_dt))

    bpack = np.zeros((128, 5), dtype=np.float32)
    bpack[:, 0:2] = b1.reshape(2, 128).T
    bpack[:, 2] = b2
    bpack[0:64, 3] = b3
    bpack[0:10, 4] = b4

    shared = {"wpack": wpack, "bpack": bpack, **w1pieces}
    in_maps = []
    for m in range(N_CORES):
        xc = x[m * BC : (m + 1) * BC]
        xt = np.ascontiguousarray(
            xc.reshape(NT, NB, NKC, KC).transpose(0, 3, 2, 1)
        ).astype(_np_dt(l1_dt))
        d = dict(shared)
        for t in range(NT):
            for i, (c0, c1) in enumerate(X_SPLITS[t]):
                d[f"xp{t}_{i}"] = np.ascontiguousarray(
                    xt[t, :, c0:c1, :].reshape(KC, (c1 - c0) * NB)
                )
        in_maps.append(d)
    return in_maps



_PROGRAM = None


def _get_program():
    global _PROGRAM
    if _PROGRAM is None:
        _PROGRAM = build_program()
    return _PROGRAM


def kernel(x, conv_w, w1, b1, w2, b2, w3, b3, w4, b4):
    from concourse import bass_utils

    args = [x, conv_w, w1, b1, w2, b2, w3, b3, w4, b4]
    x, conv_w, w1, b1, w2, b2, w3, b3, w4, b4 = [
        np.asarray(a, dtype=np.float32) for a in args
    ]
    nc = _get_program()
    in_maps = prepare_inputs(x, conv_w, w1, b1, w2, b2, w3, b3, w4, b4)
    res = bass_utils.run_bass_kernel_spmd(nc, in_maps, list(range(N_CORES)))
    out = np.concatenate(
        [
            res.results[m]["outT"].transpose(0, 2, 1).reshape(BC, 10)
            for m in range(N_CORES)
        ],
        axis=0,
    )
    return out.astype(np.float32)
